# revision 33
# baseline (speedup 1.0000x reference)
"""Viterbi decode (CRF) kernel for Trainium2, data-parallel over batch on 8 cores.

Split-direction Viterbi: a forward max-plus pass over positions 0..M and a
backward pass over S-1..M run concurrently (two independent carry chains), then
the best meeting tag at M is picked and TWO independent backtrace chains walk
outward (M->0 and M->S-1), interleaved to hide each other's latency.

The per-step broadcast-add + segmented max-reduce is fused into ONE custom DVE
instruction (ADD_SEGMAX): a MAX-scan over (carry[b,q] + trans[q,c]) whose
running value is re-seeded at each 32-element page boundary via a hand-built
SUB_DIM_DONE step state; the per-page max is read back from the last element
of each page. The feat-add runs on GPSIMD, off the DVE critical path.

The backtrace recomputes the argmax only along the traced path: the needed
trans column/row is selected by 4 concurrent 32x32 PE matmuls on a one-hot,
then a fused add+max custom DVE op (ADD_MAXREDUCE) + MaxIndex give the tag.
"""

import dataclasses as _dc
import sys

sys.path.insert(0, "/opt/trn_rl_repo")

import numpy as np

from concourse import bass, mybir
from concourse import dve_ops as _dve_ops
from concourse.dve_ops import DveOp, _COMPILE_CACHE
from concourse.dve_spec import (
    AluOp as _AluOp,
    Latch as _Latch,
    MaxNeg as _MaxNeg,
    N_LANES as _N_LANES,
    N_STAGES as _N_STAGES,
    Scan as _Scan,
    Spec as _Spec,
    Src0 as _Src0,
    Src1 as _Src1,
    Trigger as _Trigger,
    _assemble,
    _build_placement,
    _build_state_machine,
    _collect,
    _hoist_stream_invariant_ops,
    _Stage,
    _validate_body,
    lower as _lower,
    scan as _scan,
)
from concourse.dve_uop import DveOpSpec
from concourse.tile import TileContext

F32 = mybir.dt.float32
I32 = mybir.dt.int32
U32 = mybir.dt.uint32

B_LOC = 128  # batch rows per core
T = 32  # tags
N_CORES = 8

# ---------------------------------------------------------------------------
# Custom DVE ops
# ---------------------------------------------------------------------------


def _segmax_ref(in0, in1, c0, c1, c2):
    x = in0.astype(np.float32) + in1.astype(np.float32).reshape(in0.shape)
    return np.maximum.accumulate(x, axis=-1)


def _addmax_ref(in0, in1, c0, c1, c2):
    x = in0.astype(np.float32) + in1.astype(np.float32)
    acc = x.reshape(x.shape[0], -1).max(axis=-1, keepdims=True)
    return x, acc


def _lower_segmax(spec, ver):
    """Stock [seed, steady] machine plus a SUB_DIM_DONE step state that
    re-seeds the MAX-scan (max(-FLT_MAX, expr) = expr) while consuming the
    first element of each page -> per-page segmented running max."""
    n_lanes, n_stages = _N_LANES[ver], _N_STAGES[ver]
    _validate_body(spec, ver)
    spec = _hoist_stream_invariant_ops(spec)
    scans = _collect(spec.body, _Scan)
    latches = _collect(spec.body, _Latch)
    assert len(scans) == 1 and not latches
    p = _build_placement(spec, scans, n_stages, n_lanes)
    states = _build_state_machine(spec, scans, latches, p)
    assert len(states) == 2, f"expected [seed, steady], got {len(states)}"
    seed, steady = states
    sc = scans[0]
    d = p.node_stage[sc]
    steady2 = _dc.replace(
        steady,
        trigger=(_Trigger.SRC_TENSOR_DONE, _Trigger.SUB_DIM_DONE, _Trigger.NONE),
        next=(0, 2, 0),
    )
    step = _dc.replace(
        steady,
        overrides={**steady.overrides, d: _Stage(_AluOp.MAX, _MaxNeg, sc.expr)},
        trigger=(_Trigger.SRC_TENSOR_DONE, _Trigger.SUB_DIM_DONE, _Trigger.COUNT),
        next=(0, 2, 1),
        repeat=1,
    )
    out = [_assemble(s) for s in (seed, steady2, step)]
    for u in out:
        u.validate(ver)
    return out


@_dc.dataclass(frozen=True)
class _HandOp(DveOp):
    """DveOp compiled by a custom lowering; sha pin skipped (computed live)."""

    def compile(self, ver):
        key = (self.name, ver)
        if (r := _COMPILE_CACHE.get(key)) is not None:
            return r
        uops = (
            _lower_segmax(self.spec, ver)
            if self.name == "ANT_ADD_SEGMAX"
            else _lower(self.spec, ver=ver)
        )
        result = DveOpSpec(
            name=self.name,
            opcode=_dve_ops.get_dve_sub_opcode(self.name),
            uops=uops,
            rd1_en=True,
        )
        _COMPILE_CACHE[key] = result
        return result


ADD_SEGMAX = _HandOp(
    "ANT_ADD_SEGMAX",
    _Spec(body=_scan(_AluOp.MAX, _Src0 + _Src1, init=_MaxNeg), reference=_segmax_ref),
    subdim=True,
    uops_sha={},
)

ADD_MAXREDUCE = _HandOp(
    "ANT_ADD_MAXREDUCE",
    _Spec(body=_Src0 + _Src1, accum=_AluOp.MAX, reference=_addmax_ref),
    subdim=False,
    uops_sha={},
)


def _register(op):
    if op.name in _dve_ops._SUB_OPCODE_FOR_NAME:
        return
    _dve_ops.OPS.append(op)
    _dve_ops._SUB_OPCODE_FOR_NAME[op.name] = (
        _dve_ops._CUSTOM_DVE_ROW_BASE + len(_dve_ops.OPS) - 1
    )
    _dve_ops.CUSTOM_DVE_SPECS[op.name] = op.spec
    assert max(_dve_ops._SUB_OPCODE_FOR_NAME.values()) < 0x20


_register(ADD_SEGMAX)
_register(ADD_MAXREDUCE)

# ---------------------------------------------------------------------------
# Kernel build
# ---------------------------------------------------------------------------

NCONST = 2 * T * T + 7 * T  # transT, transB, transmmF, transmmB, start, stop, iota, iota2


def build_nc(S: int, fix_waits: bool = True):
    M = S // 2
    nc = bass.Bass()

    feats_d = nc.declare_dram_parameter("feats", [B_LOC, S, T], F32, isOutput=False)
    consts_d = nc.declare_dram_parameter("consts", [B_LOC, NCONST], F32, isOutput=False)
    path_d = nc.declare_dram_parameter("path", [B_LOC, S], I32, isOutput=True)

    add = mybir.AluOpType.add
    iseq = mybir.AluOpType.is_equal

    with TileContext(nc) as tc:
        with (
            tc.tile_pool(name="const", bufs=1) as cpool,
            tc.tile_pool(name="featp", bufs=1) as fpool,
            tc.tile_pool(name="work", bufs=1) as wpool,
            tc.tile_pool(name="scr", bufs=2) as spool,
            tc.tile_pool(name="psum", bufs=2, space="PSUM") as ppool,
        ):
            consts_t = cpool.tile([B_LOC, NCONST], F32)
            nc.sync.dma_start(out=consts_t[:], in_=consts_d[:])
            o = 0
            transT3 = consts_t[:, o : o + T * T].rearrange("p (c q) -> p c q", q=T)
            o += T * T
            transB3 = consts_t[:, o : o + T * T].rearrange("p (m r) -> p m r", r=T)
            o += T * T
            transmmF = consts_t[:, o : o + T]
            o += T
            transmmB = consts_t[:, o : o + T]
            o += T
            start_t = consts_t[:, o : o + T]
            o += T
            stop_t = consts_t[:, o : o + T]
            o += T
            iota_t = consts_t[:, o : o + T]
            o += T
            iota2_t = consts_t[:, o : o + 2 * T]

            # ft arrives in interleaved lo/hi chunks on the ACT engine's DMA
            # queue (serial per queue, concurrent with the consts DMA on SP's)
            # so both passes can start ~20us before the full tensor lands.
            ft = fpool.tile([B_LOC, S, T], F32)
            NCH = 4
            ck = S // (2 * NCH) if S >= 2 * NCH else S
            if ck < S:
                for k in range(NCH):
                    lo = k * ck
                    nc.scalar.dma_start(
                        out=ft[:, lo : lo + ck, :], in_=feats_d[:, lo : lo + ck, :]
                    )
                    hi = S - (k + 1) * ck
                    nc.scalar.dma_start(
                        out=ft[:, hi : hi + ck, :], in_=feats_d[:, hi : hi + ck, :]
                    )
            else:
                nc.scalar.dma_start(out=ft[:], in_=feats_d[:])

            # Interleaved state: carF[i] at slot 2(M-i) (even), vplus_{M+1+t}
            # at slot 2t+3 (odd) — backtrace round t reads the contiguous
            # pair (2t+2, 2t+3) = [carF, vplus] with one AP.
            st = wpool.tile([B_LOC, 2 * M + 2, T], F32)
            path_t = wpool.tile([B_LOC, S], I32)

            def carF(i):
                return st[:, 2 * (M - i), :]

            def vplus_slot(pos):  # pos = M+1+t -> slot 2t+3
                return st[:, 2 * (pos - M - 1) + 3, :]

            # one DMA-wait touch per (engine, DMA) pair; later consumers
            # inherit via engine program order.
            tt0 = wpool.tile([B_LOC, 1], F32, tag="touch0")
            nc.vector.tensor_copy(tt0[:], consts_t[:, 0:1])
            tt1 = wpool.tile([B_LOC, 1], F32, tag="touch1")
            nc.vector.tensor_copy(tt1[:], ft[:, 0, 0:1])
            tt2 = wpool.tile([B_LOC, 1], F32, tag="touch2")
            nc.gpsimd.tensor_copy(tt2[:], consts_t[:, 0:1])
            tt3 = wpool.tile([B_LOC, 1], F32, tag="touch3")
            nc.gpsimd.tensor_copy(tt3[:], ft[:, 0, 0:1])
            tt4 = wpool.tile([B_LOC, 1], F32, tag="touch4")
            nc.gpsimd.tensor_copy(tt4[:], ft[:, S - 1, 0:1])
            Fp0 = ppool.tile([B_LOC, T], F32, tag="Fpdummy")
            nc.tensor.matmul(
                Fp0[0:32, :],
                transmmF[0:32, :],
                transmmF[0:32, :],
                start=True,
                stop=True,
                tile_position=(0, 0),
            )

            # ---------------- forward + backward passes ----------------
            nc.vector.tensor_tensor(carF(0), ft[:, 0, :], start_t, op=add)
            # slot 2M+1 is never written (no vplus for the last F-only
            # backtrace step); zero it so the pair-read is defined.
            nc.vector.memset(st[:, 2 * M + 1, :], 0.0)

            def seg_add_max(scr, trans3, vec):
                vb = vec.unsqueeze(1).broadcast_to([B_LOC, T, T])
                nc.vector._custom_dve(ADD_SEGMAX, out=scr[:], in0=trans3, in1=vb)

            # round r: segB_r consumes vplus_{S-r} -> b_{S-1-r};
            #          segF_r consumes carF[r-1] -> carF[r]
            prevB = None  # scratchB holding b_{S-r} page-maxes at [:, :, T-1]
            for r in range(1, M + 1):
                if ck < S and r > 1 and (r - 1) % ck == 0:
                    # next ft lo/hi chunk pair: one Pool-side DMA-wait touch each
                    k = (r - 1) // ck
                    ttl = wpool.tile([B_LOC, 1], F32, tag=f"tchlo{k}")
                    nc.gpsimd.tensor_copy(ttl[:], ft[:, k * ck, 0:1])
                    tth = wpool.tile([B_LOC, 1], F32, tag=f"tchhi{k}")
                    nc.gpsimd.tensor_copy(tth[:], ft[:, S - (k + 1) * ck, 0:1])
                if r <= M - 1:
                    # featB: vplus_{S-r} = b_{S-r} + ft[S-r]
                    vp = vplus_slot(S - r)
                    bprev = stop_t if prevB is None else prevB[:, :, T - 1]
                    nc.gpsimd.tensor_tensor(vp, bprev, ft[:, S - r, :], op=add)
                    scrB = spool.tile([B_LOC, T, T], F32, tag="scrB")
                    seg_add_max(scrB, transB3, vp)
                    prevB = scrB
                scrF = spool.tile([B_LOC, T, T], F32, tag="scrF")
                seg_add_max(scrF, transT3, carF(r - 1))
                nc.gpsimd.tensor_tensor(
                    carF(r), scrF[:, :, T - 1], ft[:, r, :], op=add
                )

            # ---------------- stitch ----------------
            # MaxIndex writes its 8-wide result straight into pathW[:, pos, :];
            # element 0 is the argmax. One strided copy at the end compacts
            # pathW[:, :, 0] into path_t — no per-step path copies.
            pathW = wpool.tile([B_LOC, S, 8], U32)
            tot = wpool.tile([B_LOC, T], F32)
            mxS = wpool.tile([B_LOC, 1], F32)
            nc.vector._custom_dve(
                ADD_MAXREDUCE,
                out=tot[:],
                in0=carF(M),
                in1=prevB[:, :, T - 1],
                accum_out=mxS[:],
            )
            nc.vector.max_index(
                pathW[:, M, :], mxS[:, 0:1].broadcast_to([B_LOC, 8]), tot[:]
            )
            idxS = pathW[:, M, :]

            # ---------------- backtrace (two interleaved chains) ----------------
            ohFB = wpool.tile([B_LOC, 2 * T], F32)
            ohTFB = wpool.tile([B_LOC, 2 * T], F32)

            def oh_transpose_fb(idx_view):
                # one is_equal + one transpose cover BOTH chains: cols 0:T are
                # the F one-hot, T:2T the B one-hot; StreamTranspose acts on
                # each 32x32 block independently.
                nc.vector.tensor_tensor(
                    ohFB[:].rearrange("p (a b) -> p a b", b=T),
                    iota2_t.rearrange("p (a b) -> p a b", b=T),
                    idx_view,
                    op=iseq,
                )
                nc.vector.transpose(ohTFB[:], ohFB[:])

            def pe_select_fb():
                # page 0 (cols 0:T of ohTFB) = F chain vs transmmF,
                # page 1 (cols T:2T) = B chain vs transmmB.
                Fp = ppool.tile([B_LOC, 2, T], F32, tag="FpFB")
                for half, transmm in ((0, transmmF), (1, transmmB)):
                    for k in range(4):
                        nc.tensor.matmul(
                            Fp[32 * k : 32 * k + 32, half, :],
                            ohTFB[
                                32 * k : 32 * k + 32,
                                half * T : half * T + T,
                            ],
                            transmm[32 * k : 32 * k + 32, :],
                            start=True,
                            stop=True,
                            tile_position=(32 * k, 32 * k),
                        )
                return Fp

            # Per round, ONE 2-page ADD_SEGMAX computes both chains'
            # (state + selected-trans) running maxes; the running max doubles
            # as MaxIndex input (the first position reaching the final max IS
            # the first argmax). in1 reads the PSUM pair through a flat
            # [P, 2T] view (TTSS struct — STT src1 cannot stream PSUM).
            # F chain: steps t=0..M-1 at position i=M-t (writes path[i-1]).
            # B chain: t=0..M-2 at i=M+t (writes path[i+1]).
            def idx_pair_view(posF, posB):
                if posB == posF:
                    return (
                        pathW[:, posF, 0:1].unsqueeze(1).broadcast_to([B_LOC, 2, T])
                    )
                return pathW[:, posF : posB + 1 : posB - posF, 0:1].broadcast_to(
                    [B_LOC, 2, T]
                )

            oh_transpose_fb(idxS[:, 0:1].unsqueeze(1).broadcast_to([B_LOC, 2, T]))
            Fp = pe_select_fb()
            for t in range(M):
                seg = spool.tile([B_LOC, 2, T], F32, tag="segFB")
                nc.vector._custom_dve(
                    ADD_SEGMAX,
                    out=seg[:],
                    in0=st[:, 2 * t + 2 : 2 * t + 4, :],
                    in1=Fp[:].rearrange("p a b -> p (a b)"),
                )
                nc.vector.max_index(
                    pathW[:, M - t - 1, :],
                    seg[:, 0, T - 1 : T].broadcast_to([B_LOC, 8]),
                    seg[:, 0, :],
                )
                if t <= M - 2:
                    nc.vector.max_index(
                        pathW[:, M + t + 1, :],
                        seg[:, 1, T - 1 : T].broadcast_to([B_LOC, 8]),
                        seg[:, 1, :],
                    )
                if t + 1 <= M - 2:
                    # next round's one-hots: F tag at path[M-t-1], B at path[M+t+1]
                    oh_transpose_fb(idx_pair_view(M - t - 1, M + t + 1))
                    Fp = pe_select_fb()
                elif t + 1 == M - 1:
                    # B chain is done; only the F half matters (the B half
                    # reads the F tag too — harmless, its MaxIndex is skipped).
                    oh_transpose_fb(idx_pair_view(M - t - 1, M - t - 1))
                    Fp = pe_select_fb()

            nc.vector.tensor_copy(path_t[:], pathW[:, :, 0])
            nc.sync.dma_start(out=path_d[:], in_=path_t[:])

    mybir.codegen_inst_isa_subclasses(nc)
    if fix_waits:
        _strip_redundant_pe_waits(nc)
    return nc


def _strip_redundant_pe_waits(nc):
    """Walrus encodes at most one sync-wait per compute instruction.

    1. Merge multiple waits on the same semaphore to the max value.
    2. Split multi-wait drains into chains of single-wait drains.
    3. Drop a non-DVE wait on a DVE-waiting instruction when some DVE
       instruction with completion tick <= the DVE wait value already waited
       on that semaphore >= the required value (transitive implication that
       Tile doesn't minimize across procs)."""
    f = nc.m.functions[0]
    insts = [i for blk in f.blocks for i in blk.instructions]

    from concourse import mybir as _mybir
    import copy as _copy

    # 1. same-sem merge
    for inst in insts:
        si = inst.sync_info
        if si is None or not si.on_wait or len(si.on_wait) <= 1:
            continue
        best = {}
        for w in si.on_wait:
            k = w.ant_name
            if k not in best or w.wait_value > best[k].wait_value:
                best[k] = w
        if len(best) < len(si.on_wait):
            inst.sync_info = _mybir.SyncInfo(
                on_wait=list(best.values()), on_update=list(si.on_update or [])
            )

    # cumulative: after the k-th E-sem increment, the largest value of each
    # other semaphore that engine E has (transitively) waited on so far.
    def prefix(name):
        return name.split("_")[0]

    tick = {}  # sem prefix -> increments so far
    cur_max = {}  # engine prefix -> {other sem prefix -> max waited value}
    observed = {}  # (engine prefix, other prefix) -> [(tick, maxval)]
    for inst in insts:
        si = inst.sync_info
        if si is None:
            continue
        eng = str(inst.engine).split(".")[-1]
        cm = cur_max.setdefault(eng, {})
        for w in si.on_wait or []:
            if w.ant_name:
                p = prefix(w.ant_name)
                if p != eng:
                    cm[p] = max(cm.get(p, 0), w.wait_value)
        for u in si.on_update or []:
            if u.ant_name:
                p = prefix(u.ant_name)
                tick[p] = tick.get(p, 0) + u.update_value
                if p == eng:
                    for q, v in cm.items():
                        observed.setdefault((p, q), []).append((tick[p], v))

    def implied(via_prefix, via_val, other_name, other_val):
        """True when "via-sem >= via_val" transitively implies
        "other-sem >= other_val": the via engine had waited on other-sem
        >= other_val by the time it made its via_val-th increment."""
        p = prefix(other_name)
        best = 0
        for k, v in observed.get((via_prefix, p), []):
            if k <= via_val:
                best = max(best, v)
        return best >= other_val

    # 2. split multi-wait drains
    for blk in f.blocks:
        new_list = []
        for inst in blk.instructions:
            si = inst.sync_info
            if (
                type(inst).__name__ == "InstDrain"
                and si is not None
                and si.on_wait
                and len(si.on_wait) > 1
            ):
                waits = list(si.on_wait)
                for k, w in enumerate(waits[:-1]):
                    clone = _copy.copy(inst)
                    clone.name = f"{inst.name}-w{k}"
                    clone.sync_info = _mybir.SyncInfo(on_wait=[w], on_update=[])
                    new_list.append(clone)
                inst.sync_info = _mybir.SyncInfo(
                    on_wait=[waits[-1]], on_update=list(si.on_update or [])
                )
            new_list.append(inst)
        blk.instructions[:] = new_list

    # 3. transitivity strip: for each wait, check whether one of the OTHER
    # waits on the instruction already implies it; drop implied waits.
    for inst in insts:
        si = inst.sync_info
        if si is None or not si.on_wait or len(si.on_wait) <= 1:
            continue
        waits = list(si.on_wait)
        keep = []
        for i, w in enumerate(waits):
            redundant = any(
                implied(prefix(v.ant_name), v.wait_value, w.ant_name, w.wait_value)
                for j, v in enumerate(waits)
                if j != i and (v in keep or j > i)
            )
            if not redundant:
                keep.append(w)
        if len(keep) < len(waits):
            inst.sync_info = _mybir.SyncInfo(
                on_wait=keep, on_update=list(si.on_update or [])
            )

    remaining = [
        (i.name, type(i).__name__, [(w.ant_name, w.wait_value) for w in i.sync_info.on_wait])
        for i in insts
        if i.sync_info
        and i.sync_info.on_wait
        and len(i.sync_info.on_wait) > 1
        and type(i).__name__
        not in ("InstDrain", "InstEventSemaphore", "InstISA", "InstCall")
    ]
    if remaining:
        raise RuntimeError(f"unresolvable multi-wait instructions: {remaining[:5]}")


def _make_const_inputs(transitions, start_transitions, stop_transitions):
    transitions = np.asarray(transitions, dtype=np.float32)
    start = np.asarray(start_transitions, dtype=np.float32)
    stop = np.asarray(stop_transitions, dtype=np.float32)
    consts = np.zeros((B_LOC, NCONST), dtype=np.float32)
    o = 0
    consts[:, o : o + T * T] = transitions.T.reshape(1, T * T)  # [c*T+q] = trans[q,c]
    o += T * T
    consts[:, o : o + T * T] = transitions.reshape(1, T * T)  # [m*T+r] = trans[m,r]
    o += T * T
    consts[:, o : o + T] = np.tile(transitions.T, (4, 1))  # transmmF[p,f]=trans[f,p%32]
    o += T
    consts[:, o : o + T] = np.tile(transitions, (4, 1))  # transmmB[p,f]=trans[p%32,f]
    o += T
    consts[:, o : o + T] = start[None, :]
    o += T
    consts[:, o : o + T] = stop[None, :]
    o += T
    consts[:, o : o + T] = np.arange(T, dtype=np.float32)[None, :]
    o += T
    consts[:, o : o + 2 * T] = np.tile(np.arange(T, dtype=np.float32), 2)[None, :]
    return {"consts": consts}


class Runner:
    """Compile once, keep inputs device-resident, execute repeatedly."""

    def __init__(self, nc, n_cores=N_CORES):
        import jax
        from jax.sharding import Mesh, PartitionSpec, NamedSharding
        from jax.experimental.shard_map import shard_map
        from concourse import bass2jax

        self.jax = jax
        bass2jax.install_neuronx_cc_hook()

        partition_name = (
            nc.partition_id_tensor.name if nc.partition_id_tensor else None
        )
        in_names, out_names, out_avals, zero_outs = [], [], [], []
        for alloc in nc.m.functions[0].allocations:
            if not isinstance(alloc, mybir.MemoryLocationSet):
                continue
            name = alloc.memorylocations[0].name
            if alloc.kind == "ExternalInput":
                if name != partition_name:
                    in_names.append(name)
            elif alloc.kind == "ExternalOutput":
                out_names.append(name)
                shape = tuple(alloc.tensor_shape)
                dtype = mybir.dt.np(alloc.dtype)
                out_avals.append(jax.core.ShapedArray(shape, dtype))
                zero_outs.append(np.zeros((n_cores * shape[0], *shape[1:]), dtype))
        n_params = len(in_names)
        all_names = in_names + out_names
        if partition_name is not None:
            all_names = all_names + [partition_name]

        def _body(*args):
            operands = list(args)
            if partition_name is not None:
                operands.append(bass2jax.partition_id_tensor())
            outs = bass2jax._bass_exec_p.bind(
                *operands,
                out_avals=tuple(out_avals),
                in_names=tuple(all_names),
                out_names=tuple(out_names),
                lowering_input_output_aliases=(),
                sim_require_finite=True,
                sim_require_nnan=True,
                nc=nc,
            )
            return tuple(outs)

        self._body = _body
        devices = jax.devices()[:n_cores]
        assert len(devices) == n_cores
        self.mesh = Mesh(np.asarray(devices), ("core",))
        in_specs = (PartitionSpec("core"),) * (n_params + len(out_names))
        out_specs = (PartitionSpec("core"),) * len(out_names)
        self.sharded = jax.jit(
            shard_map(
                _body,
                mesh=self.mesh,
                in_specs=in_specs,
                out_specs=out_specs,
                check_rep=False,
            ),
            donate_argnums=tuple(range(n_params, n_params + len(out_names))),
            keep_unused=True,
        )
        self.sharding = NamedSharding(self.mesh, PartitionSpec("core"))
        self.in_names = in_names
        self.out_names = out_names
        self.out_avals = out_avals
        self.zero_outs = zero_outs
        self.n_cores = n_cores
        self.dev_in = None

    def set_inputs(self, in_maps):
        concat = [
            np.concatenate([np.asarray(m[name]) for m in in_maps], axis=0)
            for name in self.in_names
        ]
        self.dev_in = [self.jax.device_put(a, self.sharding) for a in concat]

    def execute(self):
        outs = self.sharded(*self.dev_in, *[z.copy() for z in self.zero_outs])
        outs = self.jax.block_until_ready(outs)
        return {
            name: np.asarray(outs[i]).reshape(
                self.n_cores, *self.out_avals[i].shape
            )
            for i, name in enumerate(self.out_names)
        }

    def make_chained(self, n_chain):
        """Callable dispatching the NEFF n_chain times, each execution's
        outputs threaded in as the next one's output-seed operands (data
        dependency serializes them on device); blocks once at the end.
        Wall-time slope over n_chain isolates on-device execution time from
        per-call host/RPC overhead."""
        import jax
        from jax.experimental.shard_map import shard_map
        from jax.sharding import PartitionSpec

        n_params = len(self.in_names)
        in_specs = (PartitionSpec("core"),) * (n_params + len(self.out_names))
        out_specs = (PartitionSpec("core"),) * len(self.out_names)
        fn = jax.jit(
            shard_map(
                self._body,
                mesh=self.mesh,
                in_specs=in_specs,
                out_specs=out_specs,
                check_rep=False,
            ),
            keep_unused=True,
        )
        dev_zeros = [self.jax.device_put(z, self.sharding) for z in self.zero_outs]

        def run():
            outs = tuple(dev_zeros)
            for _ in range(n_chain):
                outs = fn(*self.dev_in, *outs)
            return self.jax.block_until_ready(outs)

        return run


_RUNNER_CACHE = {}


def _get_runner(S, kind="main"):
    key = (S, kind)
    if key not in _RUNNER_CACHE:
        nc = build_nc(S) if kind == "main" else build_noop_nc(S)
        _RUNNER_CACHE[key] = Runner(nc)
    return _RUNNER_CACHE[key]


def build_noop_nc(S):
    """Same I/O signature, near-zero device work — for launch-overhead calibration."""
    nc = bass.Bass()
    nc.declare_dram_parameter("feats", [B_LOC, S, T], F32, isOutput=False)
    consts_d = nc.declare_dram_parameter("consts", [B_LOC, NCONST], F32, isOutput=False)
    path_d = nc.declare_dram_parameter("path", [B_LOC, S], I32, isOutput=True)
    with TileContext(nc) as tc:
        with tc.tile_pool(name="w", bufs=1) as pool:
            t = pool.tile([B_LOC, T], F32)
            nc.sync.dma_start(out=t[:], in_=consts_d[:, 0:T])
            ti = pool.tile([B_LOC, T], I32)
            nc.vector.tensor_copy(ti[:], t[:])
            nc.sync.dma_start(out=path_d[:, 0:T], in_=ti[:])
    _strip_redundant_pe_waits(nc)
    return nc


def _in_maps_for(feats, transitions, start_transitions, stop_transitions, n_cores):
    consts = _make_const_inputs(transitions, start_transitions, stop_transitions)
    in_maps = []
    for c in range(n_cores):
        m = dict(consts)
        m["feats"] = np.ascontiguousarray(feats[c * B_LOC : (c + 1) * B_LOC])
        in_maps.append(m)
    return in_maps


def run_on_cores(feats, transitions, start_transitions, stop_transitions, trace=False):
    feats = np.asarray(feats, dtype=np.float32)
    B, S, T_ = feats.shape
    assert T_ == T and B % B_LOC == 0
    n_cores = B // B_LOC
    runner = _get_runner(S)
    runner.set_inputs(
        _in_maps_for(feats, transitions, start_transitions, stop_transitions, n_cores)
    )
    out = runner.execute()["path"]
    return out.reshape(B, S).astype(np.int32), None


def kernel(feats, tags, transitions, start_transitions, stop_transitions):
    out, _ = run_on_cores(feats, transitions, start_transitions, stop_transitions)
    return out


# revision 34
# speedup vs baseline: 1.3305x; 1.3305x over previous
"""Viterbi decode (CRF) kernel for Trainium2, data-parallel over batch on 8 cores.

Split-direction Viterbi: a forward max-plus pass over positions 0..M and a
backward pass over S-1..M run concurrently (two independent carry chains), then
the best meeting tag at M is picked and TWO independent backtrace chains walk
outward (M->0 and M->S-1), interleaved to hide each other's latency.

The per-step broadcast-add + segmented max-reduce is fused into ONE custom DVE
instruction (ADD_SEGMAX): a MAX-scan over (carry[b,q] + trans[q,c]) whose
running value is re-seeded at each 32-element page boundary via a hand-built
SUB_DIM_DONE step state; the per-page max is read back from the last element
of each page. The feat-add runs on GPSIMD, off the DVE critical path.

The backtrace recomputes the argmax only along the traced path: the needed
trans column/row is selected by 4 concurrent 32x32 PE matmuls on a one-hot,
then a fused add+max custom DVE op (ADD_MAXREDUCE) + MaxIndex give the tag.
"""

import dataclasses as _dc
import sys

sys.path.insert(0, "/opt/trn_rl_repo")

import numpy as np

from concourse import bass, mybir
from concourse import dve_ops as _dve_ops
from concourse.dve_ops import DveOp, _COMPILE_CACHE
from concourse.dve_spec import (
    AluOp as _AluOp,
    Latch as _Latch,
    MaxNeg as _MaxNeg,
    N_LANES as _N_LANES,
    N_STAGES as _N_STAGES,
    Scan as _Scan,
    Spec as _Spec,
    Src0 as _Src0,
    Src1 as _Src1,
    Trigger as _Trigger,
    _assemble,
    _build_placement,
    _build_state_machine,
    _collect,
    _hoist_stream_invariant_ops,
    _Stage,
    _validate_body,
    lower as _lower,
    scan as _scan,
)
from concourse.dve_uop import DveOpSpec
from concourse.tile import TileContext

F32 = mybir.dt.float32
I32 = mybir.dt.int32
U32 = mybir.dt.uint32

B_LOC = 128  # batch rows per core
T = 32  # tags
N_CORES = 8

# ---------------------------------------------------------------------------
# Custom DVE ops
# ---------------------------------------------------------------------------


def _segmax_ref(in0, in1, c0, c1, c2):
    x = in0.astype(np.float32) + in1.astype(np.float32)
    return np.maximum.accumulate(x, axis=-1)


def _addmax_ref(in0, in1, c0, c1, c2):
    x = in0.astype(np.float32) + in1.astype(np.float32)
    acc = x.reshape(x.shape[0], -1).max(axis=-1, keepdims=True)
    return x, acc


def _lower_segmax(spec, ver):
    """Stock [seed, steady] machine plus a SUB_DIM_DONE step state that
    re-seeds the MAX-scan (max(-FLT_MAX, expr) = expr) while consuming the
    first element of each page -> per-page segmented running max."""
    n_lanes, n_stages = _N_LANES[ver], _N_STAGES[ver]
    _validate_body(spec, ver)
    spec = _hoist_stream_invariant_ops(spec)
    scans = _collect(spec.body, _Scan)
    latches = _collect(spec.body, _Latch)
    assert len(scans) == 1 and not latches
    p = _build_placement(spec, scans, n_stages, n_lanes)
    states = _build_state_machine(spec, scans, latches, p)
    assert len(states) == 2, f"expected [seed, steady], got {len(states)}"
    seed, steady = states
    sc = scans[0]
    d = p.node_stage[sc]
    steady2 = _dc.replace(
        steady,
        trigger=(_Trigger.SRC_TENSOR_DONE, _Trigger.SUB_DIM_DONE, _Trigger.NONE),
        next=(0, 2, 0),
    )
    step = _dc.replace(
        steady,
        overrides={**steady.overrides, d: _Stage(_AluOp.MAX, _MaxNeg, sc.expr)},
        trigger=(_Trigger.SRC_TENSOR_DONE, _Trigger.SUB_DIM_DONE, _Trigger.COUNT),
        next=(0, 2, 1),
        repeat=1,
    )
    out = [_assemble(s) for s in (seed, steady2, step)]
    for u in out:
        u.validate(ver)
    return out


@_dc.dataclass(frozen=True)
class _HandOp(DveOp):
    """DveOp compiled by a custom lowering; sha pin skipped (computed live)."""

    def compile(self, ver):
        key = (self.name, ver)
        if (r := _COMPILE_CACHE.get(key)) is not None:
            return r
        uops = (
            _lower_segmax(self.spec, ver)
            if self.name == "ANT_ADD_SEGMAX"
            else _lower(self.spec, ver=ver)
        )
        result = DveOpSpec(
            name=self.name,
            opcode=_dve_ops.get_dve_sub_opcode(self.name),
            uops=uops,
            rd1_en=True,
        )
        _COMPILE_CACHE[key] = result
        return result


ADD_SEGMAX = _HandOp(
    "ANT_ADD_SEGMAX",
    _Spec(body=_scan(_AluOp.MAX, _Src0 + _Src1, init=_MaxNeg), reference=_segmax_ref),
    subdim=True,
    uops_sha={},
)

ADD_MAXREDUCE = _HandOp(
    "ANT_ADD_MAXREDUCE",
    _Spec(body=_Src0 + _Src1, accum=_AluOp.MAX, reference=_addmax_ref),
    subdim=False,
    uops_sha={},
)


def _register(op):
    if op.name in _dve_ops._SUB_OPCODE_FOR_NAME:
        return
    _dve_ops.OPS.append(op)
    _dve_ops._SUB_OPCODE_FOR_NAME[op.name] = (
        _dve_ops._CUSTOM_DVE_ROW_BASE + len(_dve_ops.OPS) - 1
    )
    _dve_ops.CUSTOM_DVE_SPECS[op.name] = op.spec
    assert max(_dve_ops._SUB_OPCODE_FOR_NAME.values()) < 0x20


_register(ADD_SEGMAX)
_register(ADD_MAXREDUCE)

# ---------------------------------------------------------------------------
# Kernel build
# ---------------------------------------------------------------------------

NCONST = 2 * T * T + 7 * T  # transT, transB, transmmF, transmmB, start, stop, iota, iota2


def build_nc(S: int, fix_waits: bool = True):
    M = S // 2
    nc = bass.Bass()

    feats_d = nc.declare_dram_parameter("feats", [B_LOC, S, T], F32, isOutput=False)
    consts_d = nc.declare_dram_parameter("consts", [B_LOC, NCONST], F32, isOutput=False)
    path_d = nc.declare_dram_parameter("path", [B_LOC, S], I32, isOutput=True)

    add = mybir.AluOpType.add
    iseq = mybir.AluOpType.is_equal

    with TileContext(nc) as tc:
        with (
            tc.tile_pool(name="const", bufs=1) as cpool,
            tc.tile_pool(name="featp", bufs=1) as fpool,
            tc.tile_pool(name="work", bufs=1) as wpool,
            tc.tile_pool(name="scr", bufs=2) as spool,
            tc.tile_pool(name="psum", bufs=2, space="PSUM") as ppool,
        ):
            consts_t = cpool.tile([B_LOC, NCONST], F32)
            nc.sync.dma_start(out=consts_t[:], in_=consts_d[:])
            o = 0
            transT3 = consts_t[:, o : o + T * T].rearrange("p (c q) -> p c q", q=T)
            o += T * T
            transB3 = consts_t[:, o : o + T * T].rearrange("p (m r) -> p m r", r=T)
            o += T * T
            transmmF = consts_t[:, o : o + T]
            o += T
            transmmB = consts_t[:, o : o + T]
            o += T
            start_t = consts_t[:, o : o + T]
            o += T
            stop_t = consts_t[:, o : o + T]
            o += T
            iota_t = consts_t[:, o : o + T]
            o += T
            iota2_t = consts_t[:, o : o + 2 * T]

            # ft arrives in interleaved lo/hi chunks on the ACT engine's DMA
            # queue (serial per queue, concurrent with the consts DMA on SP's)
            # so both passes can start ~20us before the full tensor lands.
            ft = fpool.tile([B_LOC, S, T], F32)
            NCH = 4
            ck = S // (2 * NCH) if S >= 2 * NCH else S
            if ck < S:
                for k in range(NCH):
                    lo = k * ck
                    nc.scalar.dma_start(
                        out=ft[:, lo : lo + ck, :], in_=feats_d[:, lo : lo + ck, :]
                    )
                    hi = S - (k + 1) * ck
                    nc.scalar.dma_start(
                        out=ft[:, hi : hi + ck, :], in_=feats_d[:, hi : hi + ck, :]
                    )
            else:
                nc.scalar.dma_start(out=ft[:], in_=feats_d[:])

            carF = wpool.tile([B_LOC, M + 1, T], F32)
            vpB = wpool.tile([B_LOC, M, T], F32)  # vpB[:, j] = vplus_{M+1+j}
            path_t = wpool.tile([B_LOC, S], I32)

            # one DMA-wait touch per (engine, DMA) pair; later consumers
            # inherit via engine program order.
            tt0 = wpool.tile([B_LOC, 1], F32, tag="touch0")
            nc.vector.tensor_copy(tt0[:], consts_t[:, 0:1])
            tt1 = wpool.tile([B_LOC, 1], F32, tag="touch1")
            nc.vector.tensor_copy(tt1[:], ft[:, 0, 0:1])
            tt2 = wpool.tile([B_LOC, 1], F32, tag="touch2")
            nc.gpsimd.tensor_copy(tt2[:], consts_t[:, 0:1])
            tt3 = wpool.tile([B_LOC, 1], F32, tag="touch3")
            nc.gpsimd.tensor_copy(tt3[:], ft[:, 0, 0:1])
            tt4 = wpool.tile([B_LOC, 1], F32, tag="touch4")
            nc.gpsimd.tensor_copy(tt4[:], ft[:, S - 1, 0:1])
            Fp0 = ppool.tile([B_LOC, T], F32, tag="Fpdummy")
            nc.tensor.matmul(
                Fp0[0:32, :],
                transmmF[0:32, :],
                transmmF[0:32, :],
                start=True,
                stop=True,
                tile_position=(0, 0),
            )

            # ---------------- forward + backward passes ----------------
            nc.vector.tensor_tensor(carF[:, 0, :], ft[:, 0, :], start_t, op=add)

            def seg_add_max(scr, trans3, vec):
                vb = vec.unsqueeze(1).broadcast_to([B_LOC, T, T])
                nc.vector._custom_dve(ADD_SEGMAX, out=scr[:], in0=trans3, in1=vb)

            # round r: segB_r consumes vplus_{S-r} -> b_{S-1-r};
            #          segF_r consumes carF[r-1] -> carF[r]
            prevB = None  # scratchB holding b_{S-r} page-maxes at [:, :, T-1]
            for r in range(1, M + 1):
                if ck < S and r > 1 and (r - 1) % ck == 0:
                    # next ft lo/hi chunk pair: one Pool-side DMA-wait touch each
                    k = (r - 1) // ck
                    ttl = wpool.tile([B_LOC, 1], F32, tag=f"tchlo{k}")
                    nc.gpsimd.tensor_copy(ttl[:], ft[:, k * ck, 0:1])
                    tth = wpool.tile([B_LOC, 1], F32, tag=f"tchhi{k}")
                    nc.gpsimd.tensor_copy(tth[:], ft[:, S - (k + 1) * ck, 0:1])
                if r <= M - 1:
                    # featB: vplus_{S-r} = b_{S-r} + ft[S-r]
                    j = S - r - (M + 1)  # vpB slot for position S-r
                    bprev = stop_t if prevB is None else prevB[:, :, T - 1]
                    nc.gpsimd.tensor_tensor(vpB[:, j, :], bprev, ft[:, S - r, :], op=add)
                    scrB = spool.tile([B_LOC, T, T], F32, tag="scrB")
                    seg_add_max(scrB, transB3, vpB[:, j, :])
                    prevB = scrB
                scrF = spool.tile([B_LOC, T, T], F32, tag="scrF")
                seg_add_max(scrF, transT3, carF[:, r - 1, :])
                nc.gpsimd.tensor_tensor(
                    carF[:, r, :], scrF[:, :, T - 1], ft[:, r, :], op=add
                )

            # ---------------- stitch ----------------
            # MaxIndex writes its 8-wide result straight into pathW[:, pos, :];
            # element 0 is the argmax. One strided copy at the end compacts
            # pathW[:, :, 0] into path_t — no per-step path copies.
            pathW = wpool.tile([B_LOC, S, 8], U32)
            tot = wpool.tile([B_LOC, T], F32)
            mxS = wpool.tile([B_LOC, 1], F32)
            nc.vector._custom_dve(
                ADD_MAXREDUCE,
                out=tot[:],
                in0=carF[:, M, :],
                in1=prevB[:, :, T - 1],
                accum_out=mxS[:],
            )
            nc.vector.max_index(
                pathW[:, M, :], mxS[:, 0:1].broadcast_to([B_LOC, 8]), tot[:]
            )
            idxS = pathW[:, M, :]

            # ---------------- backtrace (two interleaved chains) ----------------
            ohFB = wpool.tile([B_LOC, 2 * T], F32)
            ohTFB = wpool.tile([B_LOC, 2 * T], F32)
            FsbF = wpool.tile([B_LOC, T], F32)
            mxF = wpool.tile([B_LOC, 1], F32)
            FsbB = wpool.tile([B_LOC, T], F32)
            mxB = wpool.tile([B_LOC, 1], F32)

            def oh_transpose_fb(idx_view):
                # one is_equal + one transpose cover BOTH chains: cols 0:T are
                # the F one-hot, T:2T the B one-hot; StreamTranspose acts on
                # each 32x32 block independently.
                nc.vector.tensor_tensor(
                    ohFB[:].rearrange("p (a b) -> p a b", b=T),
                    iota2_t.rearrange("p (a b) -> p a b", b=T),
                    idx_view,
                    op=iseq,
                )
                nc.vector.transpose(ohTFB[:], ohFB[:])

            def pe_select(cols, transmm, tag):
                Fp = ppool.tile([B_LOC, T], F32, tag=tag)
                for k in range(4):
                    nc.tensor.matmul(
                        Fp[32 * k : 32 * k + 32, :],
                        ohTFB[32 * k : 32 * k + 32, cols : cols + T],
                        transmm[32 * k : 32 * k + 32, :],
                        start=True,
                        stop=True,
                        tile_position=(32 * k, 32 * k),
                    )
                return Fp

            def add_max_idx(car_slice, Fp, Fsb, mx, pos):
                nc.vector._custom_dve(
                    ADD_MAXREDUCE, out=Fsb[:], in0=car_slice, in1=Fp[:], accum_out=mx[:]
                )
                nc.vector.max_index(
                    pathW[:, pos, :], mx[:, 0:1].broadcast_to([B_LOC, 8]), Fsb[:]
                )

            # software-pipelined: round t finishes step t (am/idx) with the
            # PSUM produced during round t-1, then preps step t+1 (oh/transp).
            # F chain: steps t=0..M-1 at position i=M-t (writes path[i-1]).
            # B chain: steps t=0..M-2 at position i=M+t (writes path[i+1]).
            oh_transpose_fb(idxS[:, 0:1].unsqueeze(1).broadcast_to([B_LOC, 2, T]))
            FpF = pe_select(0, transmmF, "FpF")
            FpB = pe_select(T, transmmB, "FpB")
            for t in range(M):
                # both adds first (independent), then both MaxIndexes: each
                # op's pipeline drain hides behind its independent sibling.
                nc.vector._custom_dve(
                    ADD_MAXREDUCE, out=FsbF[:], in0=carF[:, M - t - 1, :],
                    in1=FpF[:], accum_out=mxF[:],
                )
                if t <= M - 2:
                    nc.vector._custom_dve(
                        ADD_MAXREDUCE, out=FsbB[:], in0=vpB[:, t, :],
                        in1=FpB[:], accum_out=mxB[:],
                    )
                nc.vector.max_index(
                    pathW[:, M - t - 1, :],
                    mxF[:, 0:1].broadcast_to([B_LOC, 8]), FsbF[:],
                )
                if t <= M - 2:
                    nc.vector.max_index(
                        pathW[:, M + t + 1, :],
                        mxB[:, 0:1].broadcast_to([B_LOC, 8]), FsbB[:],
                    )
                if t + 1 <= M - 2:
                    # next round's one-hots: F tag at path[M-t-1], B at path[M+t+1]
                    step = 2 * t + 2
                    idx_view = pathW[
                        :, M - t - 1 : M + t + 2 : step, 0:1
                    ].broadcast_to([B_LOC, 2, T])
                    oh_transpose_fb(idx_view)
                    FpF = pe_select(0, transmmF, "FpF")
                    FpB = pe_select(T, transmmB, "FpB")
                elif t + 1 == M - 1:
                    # B chain is done; final F-only step uses the F half
                    idx_view = (
                        pathW[:, M - t - 1, 0:1]
                        .unsqueeze(1)
                        .broadcast_to([B_LOC, 2, T])
                    )
                    oh_transpose_fb(idx_view)
                    FpF = pe_select(0, transmmF, "FpF")

            nc.vector.tensor_copy(path_t[:], pathW[:, :, 0])
            nc.sync.dma_start(out=path_d[:], in_=path_t[:])

    mybir.codegen_inst_isa_subclasses(nc)
    if fix_waits:
        _strip_redundant_pe_waits(nc)
    return nc


def _strip_redundant_pe_waits(nc):
    """Walrus encodes at most one sync-wait per compute instruction.

    1. Merge multiple waits on the same semaphore to the max value.
    2. Split multi-wait drains into chains of single-wait drains.
    3. Drop a non-DVE wait on a DVE-waiting instruction when some DVE
       instruction with completion tick <= the DVE wait value already waited
       on that semaphore >= the required value (transitive implication that
       Tile doesn't minimize across procs)."""
    f = nc.m.functions[0]
    insts = [i for blk in f.blocks for i in blk.instructions]

    from concourse import mybir as _mybir
    import copy as _copy

    # 1. same-sem merge
    for inst in insts:
        si = inst.sync_info
        if si is None or not si.on_wait or len(si.on_wait) <= 1:
            continue
        best = {}
        for w in si.on_wait:
            k = w.ant_name
            if k not in best or w.wait_value > best[k].wait_value:
                best[k] = w
        if len(best) < len(si.on_wait):
            inst.sync_info = _mybir.SyncInfo(
                on_wait=list(best.values()), on_update=list(si.on_update or [])
            )

    # cumulative: after the k-th E-sem increment, the largest value of each
    # other semaphore that engine E has (transitively) waited on so far.
    def prefix(name):
        return name.split("_")[0]

    tick = {}  # sem prefix -> increments so far
    cur_max = {}  # engine prefix -> {other sem prefix -> max waited value}
    observed = {}  # (engine prefix, other prefix) -> [(tick, maxval)]
    for inst in insts:
        si = inst.sync_info
        if si is None:
            continue
        eng = str(inst.engine).split(".")[-1]
        cm = cur_max.setdefault(eng, {})
        for w in si.on_wait or []:
            if w.ant_name:
                p = prefix(w.ant_name)
                if p != eng:
                    cm[p] = max(cm.get(p, 0), w.wait_value)
        for u in si.on_update or []:
            if u.ant_name:
                p = prefix(u.ant_name)
                tick[p] = tick.get(p, 0) + u.update_value
                if p == eng:
                    for q, v in cm.items():
                        observed.setdefault((p, q), []).append((tick[p], v))

    def implied(via_prefix, via_val, other_name, other_val):
        """True when "via-sem >= via_val" transitively implies
        "other-sem >= other_val": the via engine had waited on other-sem
        >= other_val by the time it made its via_val-th increment."""
        p = prefix(other_name)
        best = 0
        for k, v in observed.get((via_prefix, p), []):
            if k <= via_val:
                best = max(best, v)
        return best >= other_val

    # 2. split multi-wait drains
    for blk in f.blocks:
        new_list = []
        for inst in blk.instructions:
            si = inst.sync_info
            if (
                type(inst).__name__ == "InstDrain"
                and si is not None
                and si.on_wait
                and len(si.on_wait) > 1
            ):
                waits = list(si.on_wait)
                for k, w in enumerate(waits[:-1]):
                    clone = _copy.copy(inst)
                    clone.name = f"{inst.name}-w{k}"
                    clone.sync_info = _mybir.SyncInfo(on_wait=[w], on_update=[])
                    new_list.append(clone)
                inst.sync_info = _mybir.SyncInfo(
                    on_wait=[waits[-1]], on_update=list(si.on_update or [])
                )
            new_list.append(inst)
        blk.instructions[:] = new_list

    # 3. transitivity strip: for each wait, check whether one of the OTHER
    # waits on the instruction already implies it; drop implied waits.
    for inst in insts:
        si = inst.sync_info
        if si is None or not si.on_wait or len(si.on_wait) <= 1:
            continue
        waits = list(si.on_wait)
        keep = []
        for i, w in enumerate(waits):
            redundant = any(
                implied(prefix(v.ant_name), v.wait_value, w.ant_name, w.wait_value)
                for j, v in enumerate(waits)
                if j != i and (v in keep or j > i)
            )
            if not redundant:
                keep.append(w)
        if len(keep) < len(waits):
            inst.sync_info = _mybir.SyncInfo(
                on_wait=keep, on_update=list(si.on_update or [])
            )

    remaining = [
        (i.name, type(i).__name__, [(w.ant_name, w.wait_value) for w in i.sync_info.on_wait])
        for i in insts
        if i.sync_info
        and i.sync_info.on_wait
        and len(i.sync_info.on_wait) > 1
        and type(i).__name__
        not in ("InstDrain", "InstEventSemaphore", "InstISA", "InstCall")
    ]
    if remaining:
        raise RuntimeError(f"unresolvable multi-wait instructions: {remaining[:5]}")


def _make_const_inputs(transitions, start_transitions, stop_transitions):
    transitions = np.asarray(transitions, dtype=np.float32)
    start = np.asarray(start_transitions, dtype=np.float32)
    stop = np.asarray(stop_transitions, dtype=np.float32)
    consts = np.zeros((B_LOC, NCONST), dtype=np.float32)
    o = 0
    consts[:, o : o + T * T] = transitions.T.reshape(1, T * T)  # [c*T+q] = trans[q,c]
    o += T * T
    consts[:, o : o + T * T] = transitions.reshape(1, T * T)  # [m*T+r] = trans[m,r]
    o += T * T
    consts[:, o : o + T] = np.tile(transitions.T, (4, 1))  # transmmF[p,f]=trans[f,p%32]
    o += T
    consts[:, o : o + T] = np.tile(transitions, (4, 1))  # transmmB[p,f]=trans[p%32,f]
    o += T
    consts[:, o : o + T] = start[None, :]
    o += T
    consts[:, o : o + T] = stop[None, :]
    o += T
    consts[:, o : o + T] = np.arange(T, dtype=np.float32)[None, :]
    o += T
    consts[:, o : o + 2 * T] = np.tile(np.arange(T, dtype=np.float32), 2)[None, :]
    return {"consts": consts}


class Runner:
    """Compile once, keep inputs device-resident, execute repeatedly."""

    def __init__(self, nc, n_cores=N_CORES):
        import jax
        from jax.sharding import Mesh, PartitionSpec, NamedSharding
        from jax.experimental.shard_map import shard_map
        from concourse import bass2jax

        self.jax = jax
        bass2jax.install_neuronx_cc_hook()

        partition_name = (
            nc.partition_id_tensor.name if nc.partition_id_tensor else None
        )
        in_names, out_names, out_avals, zero_outs = [], [], [], []
        for alloc in nc.m.functions[0].allocations:
            if not isinstance(alloc, mybir.MemoryLocationSet):
                continue
            name = alloc.memorylocations[0].name
            if alloc.kind == "ExternalInput":
                if name != partition_name:
                    in_names.append(name)
            elif alloc.kind == "ExternalOutput":
                out_names.append(name)
                shape = tuple(alloc.tensor_shape)
                dtype = mybir.dt.np(alloc.dtype)
                out_avals.append(jax.core.ShapedArray(shape, dtype))
                zero_outs.append(np.zeros((n_cores * shape[0], *shape[1:]), dtype))
        n_params = len(in_names)
        all_names = in_names + out_names
        if partition_name is not None:
            all_names = all_names + [partition_name]

        def _body(*args):
            operands = list(args)
            if partition_name is not None:
                operands.append(bass2jax.partition_id_tensor())
            outs = bass2jax._bass_exec_p.bind(
                *operands,
                out_avals=tuple(out_avals),
                in_names=tuple(all_names),
                out_names=tuple(out_names),
                lowering_input_output_aliases=(),
                sim_require_finite=True,
                sim_require_nnan=True,
                nc=nc,
            )
            return tuple(outs)

        self._body = _body
        devices = jax.devices()[:n_cores]
        assert len(devices) == n_cores
        self.mesh = Mesh(np.asarray(devices), ("core",))
        in_specs = (PartitionSpec("core"),) * (n_params + len(out_names))
        out_specs = (PartitionSpec("core"),) * len(out_names)
        self.sharded = jax.jit(
            shard_map(
                _body,
                mesh=self.mesh,
                in_specs=in_specs,
                out_specs=out_specs,
                check_rep=False,
            ),
            donate_argnums=tuple(range(n_params, n_params + len(out_names))),
            keep_unused=True,
        )
        self.sharding = NamedSharding(self.mesh, PartitionSpec("core"))
        self.in_names = in_names
        self.out_names = out_names
        self.out_avals = out_avals
        self.zero_outs = zero_outs
        self.n_cores = n_cores
        self.dev_in = None

    def set_inputs(self, in_maps):
        concat = [
            np.concatenate([np.asarray(m[name]) for m in in_maps], axis=0)
            for name in self.in_names
        ]
        self.dev_in = [self.jax.device_put(a, self.sharding) for a in concat]

    def execute(self):
        outs = self.sharded(*self.dev_in, *[z.copy() for z in self.zero_outs])
        outs = self.jax.block_until_ready(outs)
        return {
            name: np.asarray(outs[i]).reshape(
                self.n_cores, *self.out_avals[i].shape
            )
            for i, name in enumerate(self.out_names)
        }

    def make_chained(self, n_chain):
        """Callable dispatching the NEFF n_chain times, each execution's
        outputs threaded in as the next one's output-seed operands (data
        dependency serializes them on device); blocks once at the end.
        Wall-time slope over n_chain isolates on-device execution time from
        per-call host/RPC overhead."""
        import jax
        from jax.experimental.shard_map import shard_map
        from jax.sharding import PartitionSpec

        n_params = len(self.in_names)
        in_specs = (PartitionSpec("core"),) * (n_params + len(self.out_names))
        out_specs = (PartitionSpec("core"),) * len(self.out_names)
        fn = jax.jit(
            shard_map(
                self._body,
                mesh=self.mesh,
                in_specs=in_specs,
                out_specs=out_specs,
                check_rep=False,
            ),
            keep_unused=True,
        )
        dev_zeros = [self.jax.device_put(z, self.sharding) for z in self.zero_outs]

        def run():
            outs = tuple(dev_zeros)
            for _ in range(n_chain):
                outs = fn(*self.dev_in, *outs)
            return self.jax.block_until_ready(outs)

        return run


_RUNNER_CACHE = {}


def _get_runner(S, kind="main"):
    key = (S, kind)
    if key not in _RUNNER_CACHE:
        nc = build_nc(S) if kind == "main" else build_noop_nc(S)
        _RUNNER_CACHE[key] = Runner(nc)
    return _RUNNER_CACHE[key]


def build_noop_nc(S):
    """Same I/O signature, near-zero device work — for launch-overhead calibration."""
    nc = bass.Bass()
    nc.declare_dram_parameter("feats", [B_LOC, S, T], F32, isOutput=False)
    consts_d = nc.declare_dram_parameter("consts", [B_LOC, NCONST], F32, isOutput=False)
    path_d = nc.declare_dram_parameter("path", [B_LOC, S], I32, isOutput=True)
    with TileContext(nc) as tc:
        with tc.tile_pool(name="w", bufs=1) as pool:
            t = pool.tile([B_LOC, T], F32)
            nc.sync.dma_start(out=t[:], in_=consts_d[:, 0:T])
            ti = pool.tile([B_LOC, T], I32)
            nc.vector.tensor_copy(ti[:], t[:])
            nc.sync.dma_start(out=path_d[:, 0:T], in_=ti[:])
    _strip_redundant_pe_waits(nc)
    return nc


def _in_maps_for(feats, transitions, start_transitions, stop_transitions, n_cores):
    consts = _make_const_inputs(transitions, start_transitions, stop_transitions)
    in_maps = []
    for c in range(n_cores):
        m = dict(consts)
        m["feats"] = np.ascontiguousarray(feats[c * B_LOC : (c + 1) * B_LOC])
        in_maps.append(m)
    return in_maps


def run_on_cores(feats, transitions, start_transitions, stop_transitions, trace=False):
    feats = np.asarray(feats, dtype=np.float32)
    B, S, T_ = feats.shape
    assert T_ == T and B % B_LOC == 0
    n_cores = B // B_LOC
    runner = _get_runner(S)
    runner.set_inputs(
        _in_maps_for(feats, transitions, start_transitions, stop_transitions, n_cores)
    )
    out = runner.execute()["path"]
    return out.reshape(B, S).astype(np.int32), None


def kernel(feats, tags, transitions, start_transitions, stop_transitions):
    out, _ = run_on_cores(feats, transitions, start_transitions, stop_transitions)
    return out
